# revision 1
# baseline (speedup 1.0000x reference)
"""Channel-attention (XCA-style) kernel for TRN2, 8 NeuronCores, data-parallel
over batch (1 image per core).

Per image:
  q  = conv3x3(y, Wq')            Wq'[o,i,tap] = sum_c qdw[o,c,tap] qw[c,i]
  kv = dw3x3(conv1x1(x, Wkv))     -> k, v
  G[c,d]   = sum_n q[c,n] k[d,n]
  S        = G * t[c] / (|q_c||k_d|)   (block-diagonal per head)
  A        = softmax_d(S)
  out      = (P @ A) @ v  via C^T = A @ P^T on device

All matmuls fp16 (fp32 PSUM accumulate). Depthwise 3x3 via DVE/ACT
scale passes + DVE tree adds on a flat row layout with edge-column zeroing.
"""
import numpy as np

import concourse.bass as bass
import concourse.bacc as bacc
import concourse.mybir as mybir
import concourse.tile as tile
from concourse.masks import make_identity

F32 = mybir.dt.float32
FP16 = mybir.dt.float16

B, C, H, W = 8, 192, 128, 128
HEADS = 8
CH = C // HEADS            # 24
N = H * W                  # 16384
WP = W + 4                 # padded row stride for y (132)
HP = H + 2                 # padded rows (130)
NG = 8                     # row groups
GR = H // NG               # rows per group (16)
RT = 4                     # rows per conv tile
NT = H // RT               # conv tiles (32)

CC = [(0, 96), (96, 96)]   # head-aligned channel chunking (4 heads each)

# DW tap order: tap = (dy+1)*3 + (dx+1)
TAPS = [(dy, dx) for dy in (-1, 0, 1) for dx in (-1, 0, 1)]
DW_ACT_TAPS = {0, 3, 5, 8}   # scale ops on ACT; rest on DVE (tap1 = acc)


def build(repeat=1, use_for_i=False, parts=("q","kv","dw","gram","attn","fin")):
    nc = bacc.Bacc()
    tok_in = nc.dram_tensor("tok_in", [128, 16], F32, kind="ExternalInput")
    d_y = nc.dram_tensor("ypad", [C, (HP + 2) * WP], FP16, kind="ExternalInput")
    d_x = nc.dram_tensor("x16", [C, N], FP16, kind="ExternalInput")
    d_wq = nc.dram_tensor("wq", [3 * C, 3 * C], FP16, kind="ExternalInput")
    d_wkv = nc.dram_tensor("wkv", [C, 2 * C], FP16, kind="ExternalInput")
    d_wdw = nc.dram_tensor("wdw", [3 * 128, 16], F32, kind="ExternalInput")
    d_wp = nc.dram_tensor("wproj", [C, C], FP16, kind="ExternalInput")
    d_tv = nc.dram_tensor("tvec", [C, 16], F32, kind="ExternalInput")
    d_msk = nc.dram_tensor("smask", [C, C], F32, kind="ExternalInput")
    d_out = nc.dram_tensor("out", [C, N], F32, kind="ExternalOutput")
    d_tok = nc.dram_tensor("tok_out", [128, 16], F32, kind="ExternalOutput")
    d_vsp = nc.dram_tensor("vspill", [C, N], FP16, kind="Internal")

    with tile.TileContext(nc) as tc:
        with (
            tc.tile_pool(name="wp", bufs=1) as wp,
            tc.tile_pool(name="io", bufs=2) as io,
            tc.tile_pool(name="qt", bufs=1) as qtp,
            tc.tile_pool(name="dw", bufs=2) as dwp,
            tc.tile_pool(name="sm", bufs=1) as sm,
            tc.tile_pool(name="ps", bufs=1, space="PSUM") as ps,
        ):
            tki = sm.tile([128, 16], F32)
            nc.sync.dma_start(tki, tok_in[:, :])

            t_wq = {}
            t_wkv = {}
            t_wp_ = {}
            t_tv = {}
            t_msk = {}
            WCH = [(0, 128), (128, 128), (256, 128), (384, 128), (512, 64)]
            for k0, kn in WCH:
                t_wq[k0] = wp.tile([kn, 3 * C], FP16, name=f"wq{k0}")
                nc.sync.dma_start(t_wq[k0], d_wq[k0:k0 + kn, :])
            for c0, cn in CC:
                t_wkv[c0] = wp.tile([cn, 2 * C], FP16, name=f"wkv{c0}")
                nc.sync.dma_start(t_wkv[c0], d_wkv[c0:c0 + cn, :])
                t_wp_[c0] = wp.tile([cn, C], FP16, name=f"wp{c0}")
                nc.sync.dma_start(t_wp_[c0], d_wp[c0:c0 + cn, :])
                t_tv[c0] = wp.tile([cn, 16], F32, name=f"tv{c0}")
                nc.sync.dma_start(t_tv[c0], d_tv[c0:c0 + cn, :])
                t_msk[c0] = wp.tile([cn, C], F32, name=f"msk{c0}")
                nc.sync.dma_start(t_msk[c0], d_msk[c0:c0 + cn, :])
            t_wdw = {}
            for t in range(3):
                t_wdw[t] = wp.tile([128, 16], F32, name=f"wdw{t}")
                nc.sync.dma_start(t_wdw[t], d_wdw[t * 128:(t + 1) * 128, :])
            id32 = wp.tile([128, 128], F32, name="id32")
            make_identity(nc, id32)
            id16 = wp.tile([128, 128], FP16, name="id16")
            make_identity(nc, id16)
            ones1 = wp.tile([1, C], F32, name="ones1")
            nc.vector.memset(ones1, 1.0)

            state = {}

            def body(it=None):
                t_qT = qtp.tile([128, H * C], FP16, name="qT", tag="qT")
                qT3 = t_qT[:, :].rearrange("p (j c) -> p j c", c=C)
                qn = {c0: sm.tile([cn, 32], F32, name=f"qn{c0}", tag=f"qn{c0}")
                      for c0, cn in CC}
                kna = sm.tile([128, 8], F32, name="kna", tag="kna")
                knb = sm.tile([64, 8], F32, name="knb", tag="knb")
                pGall = ps.tile([96, 2 * C], F32, name="pGall", tag="pGall")
                pG = {0: pGall[:, 0:C], 96: pGall[:, C:2 * C]}
                kv1 = {}

                def qconv_tile(t):
                    g = (RT * t) // GR
                    r0 = RT * t
                    if r0 % GR == 0:
                        # 5 K-chunk tiles over (dy, ci); local row L holds
                        # ypad2 row (16g + L + dy + 1)
                        ty = []
                        tA = io.tile([128, 18 * WP], FP16, name="y3a",
                                     tag="y3a", bufs=2)
                        nc.sync.dma_start(
                            tA, d_y[0:128, (16 * g) * WP:(16 * g + 18) * WP])
                        ty.append(tA)
                        tB = io.tile([128, 18 * WP], FP16, name="y3b",
                                     tag="y3b", bufs=2)
                        nc.sync.dma_start(
                            tB[0:64, :],
                            d_y[128:192, (16 * g) * WP:(16 * g + 18) * WP])
                        nc.sync.dma_start(
                            tB[64:128, :],
                            d_y[0:64, (16 * g + 1) * WP:(16 * g + 19) * WP])
                        ty.append(tB)
                        tC = io.tile([128, 18 * WP], FP16, name="y3c",
                                     tag="y3c", bufs=2)
                        nc.sync.dma_start(
                            tC, d_y[64:192,
                                    (16 * g + 1) * WP:(16 * g + 19) * WP])
                        ty.append(tC)
                        tD = io.tile([128, 18 * WP], FP16, name="y3d",
                                     tag="y3d", bufs=2)
                        nc.sync.dma_start(
                            tD, d_y[0:128,
                                    (16 * g + 2) * WP:(16 * g + 20) * WP])
                        ty.append(tD)
                        tE = io.tile([64, 18 * WP], FP16, name="y3e",
                                     tag="y3e", bufs=2)
                        nc.sync.dma_start(
                            tE, d_y[128:192,
                                    (16 * g + 2) * WP:(16 * g + 20) * WP])
                        ty.append(tE)
                        state["y"] = ty
                    ty = state["y"]
                    WCH = [(0, 128), (128, 128), (256, 128), (384, 128),
                           (512, 64)]
                    pq = {0: ps.tile([96, 512], F32, name="pq0", tag="pq0",
                                     bufs=2),
                          96: ps.tile([96, 512], F32, name="pq1", tag="pq1",
                                      bufs=1)}
                    for o0, on in CC:
                        nmm = 0
                        for dxi in range(3):
                            for ci in range(5):
                                src = ty[ci]
                                k0, kn = WCH[ci]
                                off = (r0 - 16 * g + 1) * WP + 2 + (dxi - 1)
                                rhs = bass.AP(
                                    tensor=src.tensor, offset=src.offset + off,
                                    ap=[src.ap[0], [WP, RT], [1, W]])
                                nc.tensor.matmul(
                                    pq[o0],
                                    t_wq[k0][:, dxi * C + o0: dxi * C + o0 + on],
                                    rhs, start=(nmm == 0), stop=(nmm == 14))
                                nmm += 1
                    for o0, on in CC:
                        qs = io.tile([on, 512], FP16, name=f"qs{o0}",
                                     tag=f"qs{o0}", bufs=2)
                        nc.scalar.copy(qs, pq[o0])
                        scr = io.tile([on, 512], FP16, name="sqscr",
                                      tag="sqscr", bufs=1)
                        nc.scalar.activation(
                            scr, qs, mybir.ActivationFunctionType.Square,
                            accum_out=qn[o0][:, t:t + 1])
                        ptr = ps.tile([128, RT * 96], FP16, name="ptrq",
                                      tag="ptp", bufs=2)
                        for j in range(RT):
                            nc.tensor.transpose(
                                ptr[:, j * 96:(j + 1) * 96],
                                qs[:, j * 128:(j + 1) * 128],
                                id16[0:96, 0:96])
                        dst = bass.AP(
                            tensor=t_qT.tensor,
                            offset=t_qT.offset + r0 * C + o0,
                            ap=[t_qT.ap[0], [C, RT], [1, on]])
                        nc.vector.tensor_copy(dst, ptr)

                def new_kv1(m, ct):
                    kt = dwp.tile([128, 18 * W + 8], FP16, name=f"kv1_{ct}",
                                  tag=f"kv1_{ct}", bufs=2)
                    kv1[(m, ct)] = kt
                    if m == 0:
                        nc.vector.memset(kt[:, 4:4 + W], 0.0)
                    if m == NG - 1:
                        nc.vector.memset(kt[:, 4 + 17 * W:4 + 18 * W], 0.0)
                    return kt

                def kvconv_tile(t):
                    r0 = RT * t
                    tx = {}
                    for c0, cn in CC:
                        tx[c0] = io.tile([cn, 512], FP16, name=f"xg{c0}",
                                         tag=f"xg{c0}", bufs=2)
                        nc.sync.dma_start(
                            tx[c0], d_x[c0:c0 + cn, 512 * t:512 * (t + 1)])
                    m = t // 4
                    for ct in range(3):
                        pkv = ps.tile([128, 512], F32, name="pkv", tag="pkv",
                                      bufs=2)
                        for i, (c0, cn) in enumerate(CC):
                            nc.tensor.matmul(
                                pkv, t_wkv[c0][:, ct * 128:(ct + 1) * 128],
                                tx[c0][:, :],
                                start=(i == 0), stop=(i == 1))
                        kt = kv1.get((m, ct))
                        if kt is None:
                            kt = new_kv1(m, ct)
                        lo = r0 - 16 * m + 1
                        nc.scalar.copy(kt[:, 4 + lo * W:4 + (lo + RT) * W], pkv)
                        if r0 % 16 == 0 and m >= 1:
                            nc.scalar.copy(
                                kv1[(m - 1, ct)][:, 4 + 17 * W:4 + 18 * W], pkv[:, 0:W])
                        if (r0 + 3) % 16 == 15 and m + 1 <= NG - 1:
                            nxt = kv1.get((m + 1, ct))
                            if nxt is None:
                                nxt = new_kv1(m + 1, ct)
                            nc.scalar.copy(nxt[:, 4:4 + W], pkv[:, 3 * W:4 * W])

                def dw_group(g):
                    kT_g = io.tile([128, GR * C], FP16, name="kTg", tag="kTg",
                                   bufs=2)
                    for ct in range(3):
                        src = kv1[(g, ct)]
                        wcol = t_wdw[ct]
                        acc = dwp.tile([128, 2048], FP16, name="dwacc",
                                       tag="dwacc", bufs=3)
                        nc.vector.tensor_scalar_mul(
                            acc, src[:, 4:4 + 16 * W], wcol[:, 1:2])
                        for tap in range(9):
                            if tap == 1:
                                continue
                            dy, dx = TAPS[tap]
                            z = dwp.tile([128, 2048], FP16, name="dwz",
                                         tag="dwz", bufs=3)
                            off = 4 + (1 + dy) * W + dx
                            sap = bass.AP(tensor=src.tensor,
                                          offset=src.offset + off,
                                          ap=[src.ap[0], [1, 2048]])
                            if tap in DW_ACT_TAPS:
                                nc.scalar.mul(z, sap, wcol[:, tap:tap + 1])
                            else:
                                nc.vector.tensor_scalar_mul(
                                    z, sap, wcol[:, tap:tap + 1])
                            if dx != 0:
                                col = 0 if dx == -1 else (W - 1)
                                eap = bass.AP(tensor=z.tensor,
                                              offset=z.offset + col,
                                              ap=[z.ap[0], [W, 16], [1, 1]])
                                nc.vector.memset(eap, 0.0)
                            nc.vector.tensor_add(acc, acc, z)
                        if ct == 0:
                            scr = dwp.tile([128, 2048], FP16, name="ksr",
                                           tag="dwz", bufs=3)
                            nc.scalar.activation(
                                scr, acc, mybir.ActivationFunctionType.Square,
                                accum_out=kna[:, g:g + 1])
                            state["kacc0"] = acc
                        elif ct == 1:
                            scr = dwp.tile([64, 2048], FP16, name="ksrb",
                                           tag="dwz", bufs=3)
                            nc.scalar.activation(
                                scr, acc[0:64, :],
                                mybir.ActivationFunctionType.Square,
                                accum_out=knb[:, g:g + 1])
                            nc.sync.dma_start(
                                d_vsp[0:64, 2048 * g:2048 * (g + 1)],
                                acc[64:128, :])
                            acc0 = state["kacc0"]
                            for s in range(4):
                                ptk = ps.tile([128, 4 * C], FP16, name="ptk",
                                              tag="ptp", bufs=2)
                                for j in range(4):
                                    b = 4 * s + j
                                    nc.tensor.transpose(
                                        ptk[:, j * C:j * C + 128],
                                        acc0[:, b * 128:(b + 1) * 128], id16)
                                    nc.tensor.transpose(
                                        ptk[:, j * C + 128:(j + 1) * C],
                                        acc[0:64, b * 128:(b + 1) * 128],
                                        id16[0:64, 0:64])
                                nc.vector.tensor_copy(
                                    kT_g[:, s * 4 * C:(s + 1) * 4 * C], ptk)
                        else:
                            nc.sync.dma_start(
                                d_vsp[64:192, 2048 * g:2048 * (g + 1)], acc)
                    if "gram" in parts:
                        for j in range(GR):
                            r = 16 * g + j
                            for c0, cn in CC:
                                nc.tensor.matmul(
                                    pG[c0], qT3[:, r, c0:c0 + cn],
                                    kT_g[:, j * C:(j + 1) * C],
                                    start=(r == 0 and c0 == 0),
                                    stop=(r == H - 1 and c0 == 96))

                for g in range(NG):
                    for t in range(4 * g, 4 * g + 4):
                        if "q" in parts:
                            qconv_tile(t)
                        if "kv" in parts:
                            kvconv_tile(t)
                    if g >= 1 and "dw" in parts:
                        dw_group(g - 1)
                if "dw" in parts:
                    dw_group(NG - 1)
                if "attn" not in parts:
                    return

                # ---------------- attention ----------------
                rq = {}
                for c0, cn in CC:
                    s = sm.tile([cn, 1], F32, name=f"qn2_{c0}", tag=f"qn2_{c0}")
                    nc.vector.tensor_reduce(s, qn[c0], axis=mybir.AxisListType.X,
                                            op=mybir.AluOpType.add)
                    nc.scalar.sqrt(s, s)
                    nc.vector.reciprocal(s, s)
                    nc.vector.tensor_mul(s, s, t_tv[c0][:, 0:1])
                    rq[c0] = s
                kn2a = sm.tile([128, 1], F32, name="kn2a", tag="kn2a")
                nc.vector.tensor_reduce(kn2a, kna, axis=mybir.AxisListType.X,
                                        op=mybir.AluOpType.add)
                kn2b = sm.tile([64, 1], F32, name="kn2b", tag="kn2b")
                nc.vector.tensor_reduce(kn2b, knb, axis=mybir.AxisListType.X,
                                        op=mybir.AluOpType.add)
                pkrow = ps.tile([1, C], F32, name="pkrow", tag="pq1", bufs=1)
                nc.tensor.transpose(pkrow[0:1, 0:128], kn2a, id32)
                nc.tensor.transpose(pkrow[0:1, 128:192], kn2b,
                                    id32[0:64, 0:64])
                krow = sm.tile([1, C], F32, name="krow", tag="krow")
                nc.scalar.copy(krow, pkrow)
                nc.scalar.sqrt(krow, krow)
                nc.vector.reciprocal(krow, krow)
                rkb = {}
                for c0, cn in CC:
                    pb = ps.tile([cn, C], F32, name=f"prkb{c0}", tag="pq0",
                                 bufs=2)
                    nc.tensor.matmul(pb, ones1[0:1, c0:c0 + cn], krow,
                                     start=True, stop=True)
                    sb_ = sm.tile([cn, C], F32, name=f"rkb{c0}", tag=f"rkb{c0}")
                    nc.scalar.copy(sb_, pb)
                    rkb[c0] = sb_
                A = {}
                for c0, cn in CC:
                    s = sm.tile([cn, C], F32, name=f"S{c0}", tag=f"S{c0}")
                    nc.vector.tensor_scalar_mul(s, pG[c0], rq[c0])
                    nc.vector.tensor_mul(s, s, rkb[c0])
                    nc.vector.tensor_add(s, s, t_msk[c0])
                    m = sm.tile([cn, 1], F32, name=f"m{c0}", tag=f"m{c0}")
                    a = sm.tile([cn, C], FP16, name=f"A{c0}", tag=f"A{c0}")
                    z = sm.tile([cn, 1], F32, name=f"z{c0}", tag=f"z{c0}")
                    nc.vector.tensor_reduce(
                        m, s, axis=mybir.AxisListType.X,
                        op=mybir.AluOpType.max)
                    nc.vector.tensor_scalar_mul(m, m, -1.0)
                    nc.scalar.activation(
                        a, s, mybir.ActivationFunctionType.Exp,
                        bias=m, scale=1.0, accum_out=z)
                    nc.vector.reciprocal(z, z)
                    nc.vector.tensor_scalar_mul(a, a, z)
                    A[c0] = a
                CT = {}
                for d0, dn in CC:
                    pc = ps.tile([dn, C], F32, name=f"pCT{d0}", tag="pkv",
                                 bufs=2)
                    for i, (c0, cn) in enumerate(CC):
                        nc.tensor.matmul(
                            pc, A[c0][:, d0:d0 + dn], t_wp_[c0],
                            start=(i == 0), stop=(i == 1))
                    ct_ = sm.tile([dn, C], FP16, name=f"CT{d0}", tag=f"CT{d0}")
                    nc.scalar.copy(ct_, pc)
                    CT[d0] = ct_

                # ---------------- final conv ----------------
                for t in range(NT):
                    vin = {}
                    for d0, dn in CC:
                        vt = io.tile([dn, 512], FP16, name=f"vin{d0}",
                                     tag=f"vin{d0}", bufs=2)
                        nc.sync.dma_start(
                            vt, d_vsp[d0:d0 + dn, 512 * t:512 * (t + 1)])
                        vin[d0] = vt
                    for o0, on in CC:
                        pf = ps.tile([on, 512], F32, name=f"pf{o0}",
                                     tag=("pq0" if o0 == 0 else "pq1"),
                                     bufs=(2 if o0 == 0 else 1))
                        for i, (d0, dn) in enumerate(CC):
                            nc.tensor.matmul(
                                pf, CT[d0][:, o0:o0 + on], vin[d0],
                                start=(i == 0), stop=(i == 1))
                        ost = io.tile([on, 512], F32, name=f"ost{o0}",
                                      tag=f"ost{o0}", bufs=2)
                        nc.scalar.copy(ost, pf)
                        nc.sync.dma_start(
                            d_out[o0:o0 + on, 512 * t:512 * (t + 1)], ost)

            if use_for_i and repeat > 1:
                with tc.For_i(0, repeat, 1) as iv:
                    body(iv)
            else:
                for it in range(repeat):
                    body(it)

            o16 = sm.tile([128, 16], F32, name="o16", tag="o16")
            nc.vector.tensor_copy(o16, tki)
            nc.sync.dma_start(d_tok[:, :], o16)

    nc.compile()
    return nc


# ---------------------------------------------------------------------------
# host-side packing
# ---------------------------------------------------------------------------

def prep_weights(kv_w, kv_dw_w, q_w, q_dw_w, proj_w, temperature):
    kv_w = np.asarray(kv_w, np.float32).reshape(2 * C, C)
    kv_dw_w = np.asarray(kv_dw_w, np.float32).reshape(2 * C, 9)
    q_w = np.asarray(q_w, np.float32).reshape(C, C)
    q_dw_w = np.asarray(q_dw_w, np.float32).reshape(C, C, 9)
    proj_w = np.asarray(proj_w, np.float32).reshape(C, C)
    temperature = np.asarray(temperature, np.float32).reshape(HEADS)

    wq = np.einsum('oct,ci->oit', q_dw_w, q_w)               # [o, i, tap]
    # wq3[(dy+1)*C + ci, (dx+1)*C + o] = Wq'[o, ci, tap(dy,dx)]
    wq3 = np.zeros((3 * C, 3 * C), np.float32)
    for dy in range(3):
        for dx in range(3):
            tap = dy * 3 + dx
            wq3[dy * C:(dy + 1) * C, dx * C:(dx + 1) * C] = wq[:, :, tap].T
    wq_lhsT = wq3
    wkv_lhsT = np.ascontiguousarray(kv_w.T)                  # [ci, co]
    wdw = np.zeros((3 * 128, 16), np.float32)
    wdw[:, :9] = kv_dw_w[:3 * 128, :9]
    wproj_T = np.ascontiguousarray(proj_w.T)                 # [c, o]
    tvec = np.zeros((C, 16), np.float32)
    tvec[:, 0] = np.repeat(temperature, CH)
    smask = np.full((C, C), -60000.0, np.float32)
    for h in range(HEADS):
        smask[h * CH:(h + 1) * CH, h * CH:(h + 1) * CH] = 0.0
    return (wq_lhsT.astype(np.float16), wkv_lhsT.astype(np.float16),
            wdw, wproj_T.astype(np.float16), tvec, smask)


def prep_image(xi, yi):
    x16 = np.asarray(xi, np.float32).reshape(C, N).astype(np.float16)
    yp = np.zeros((C, HP + 2, WP), np.float16)
    yp[:, 2:2 + H, 2:2 + W] = np.asarray(yi, np.float32).astype(np.float16)
    return x16, yp.reshape(C, (HP + 2) * WP)


_CACHE = {}


def make_in_maps(x, y, kv_w, kv_dw_w, q_w, q_dw_w, proj_w, temperature):
    x = np.asarray(x, np.float32)
    y = np.asarray(y, np.float32)
    wq, wkv, wdw, wpj, tv, smask = prep_weights(
        kv_w, kv_dw_w, q_w, q_dw_w, proj_w, temperature)
    tok = np.zeros((128, 16), np.float32)
    in_maps = []
    for b in range(B):
        x16, yp = prep_image(x[b], y[b])
        in_maps.append({
            "tok_in": tok, "ypad": yp, "x16": x16,
            "wq": wq, "wkv": wkv, "wdw": wdw, "wproj": wpj, "tvec": tv, "smask": smask,
        })
    return in_maps


def kernel(x, y, kv_w, kv_dw_w, q_w, q_dw_w, proj_w, temperature):
    in_maps = make_in_maps(x, y, kv_w, kv_dw_w, q_w, q_dw_w, proj_w,
                           temperature)
    if "nc" not in _CACHE:
        _CACHE["nc"] = build()
    nc = _CACHE["nc"]
    from concourse.bass_utils import run_bass_kernel_spmd
    res = run_bass_kernel_spmd(nc, in_maps, core_ids=list(range(B)))
    out = np.stack([res.results[b]["out"].reshape(C, H, W) for b in range(B)])
    return out.astype(np.float32)

